# revision 5
# baseline (speedup 1.0000x reference)
"""AttentionalGraphAggregation (segment softmax + weighted scatter-sum) on 8 trn2 cores.

Math (eval mode, dropout = id):
    h     = relu(x @ W1 + b1)            [N, 64]
    gate  = (h @ W2 + b2)[:, 0]          [N]
    alpha = segment_softmax(gate, index) [N]   (max-subtraction skipped: gate is
                                               tiny (|gate| < ~0.3) so exp is safe,
                                               and alpha is mathematically identical)
    t     = relu(x @ Wt + bt)            [N, 128]
    out   = segment_sum(alpha[:,None] * t, index, 8192)

Device strategy (per core; data-parallel over segments per the sharding hint):
  - Core k owns segments [1024k, 1024(k+1)); index is sorted so its nodes are a
    contiguous slice.  Host pre-transposes x (ships xT [128, M_pad]) so that a
    column-slice of xT is directly usable as the matmul stationary operand:
    out = xT_chunk.T @ W = x_chunk @ W  (natural, nodes-on-partitions output).
  - gate via the relu identity relu(u) = (u + |u|)/2 with W2 folded into W1:
        gate = 0.5*(x@(W1@w2) + sum|x@W1p| - sum|x@W1m|) + const
    where W1p/W1m are W1 columns scaled by |w2| split by sign(w2).  This keeps
    everything in one 193-wide matmul per 128-node chunk and lets the DVE do the
    reductions straight out of PSUM (abs+add reduce), with no relu_h tensor.
  - e = exp(gate) on ACT; t = relu(x@Wt) on ACT (PSUM->SBUF).
  - scatter: per chunk a one-hot matrix B[p, s] = e_p * (segloc_p == s) is built
    in ONE DVE tensor_scalar op (iota is_equal segloc, then mult by e), then
    U[win] += B.T @ [t | ones] accumulates per-window segment sums (and denoms)
    in PSUM.  Windows are 32 segments; host pads each window's nodes to a
    uniform chunk count so the SPMD program is identical across cores.
  - flush per window: out = U / (denom + 1e-16) (per-partition scalar) -> DMA.
"""

import sys

if "/opt/trn_rl_repo" not in sys.path:
    sys.path.insert(0, "/opt/trn_rl_repo")

import numpy as np

import concourse.bacc as bacc
import concourse.bass as bass
import concourse.mybir as mybir
import concourse.tile as tile
from concourse.bass_utils import run_bass_kernel_spmd

F32 = mybir.dt.float32
ALU = mybir.AluOpType
ACTF = mybir.ActivationFunctionType
AX = mybir.AxisListType

N_CORES = 8
D = 128          # feature dim (both in and out)
DH = 64          # gate hidden dim
CHUNK = 128      # nodes per matmul chunk (stationary width)
GROUP = 8        # chunks per pipeline group
WIN = 32         # segments per scatter window (B width / U partition count)
EPS = 1e-16


def _host_shard(x, index, segs):
    """Shard nodes by segment windows, pad each window to a uniform chunk count.

    Returns per-core xT [128, M_pad] (f32), segloc [128, n_chunks] (f32, -1 for
    padding), plus (C, M_pad, n_chunks, spc, nwin).
    """
    n = x.shape[0]
    spc = segs // N_CORES              # segments per core
    nwin = spc // WIN                  # windows per core
    idx = np.asarray(index)
    if idx.dtype != np.int64:
        idx = idx.astype(np.int64)
    if not np.all(idx[1:] >= idx[:-1]):
        perm = np.argsort(idx, kind="stable")
        idx = idx[perm]
        x = np.asarray(x)[perm]
    # window boundaries: windows are WIN consecutive segments
    wb = np.searchsorted(idx, np.arange(0, segs + 1, WIN))  # [n_windows_total+1]
    wcounts = np.diff(wb)
    cmax = int(np.ceil(wcounts.max() / CHUNK)) if n else 1
    C = max(GROUP, ((cmax + GROUP - 1) // GROUP) * GROUP)   # chunks per window
    m_pad = nwin * C * CHUNK
    n_chunks = nwin * C

    xs, segls = [], []
    x = np.asarray(x, dtype=np.float32)
    for k in range(N_CORES):
        xk = np.zeros((m_pad, D), np.float32)
        sk = np.full((m_pad,), -1.0, np.float32)
        for w in range(nwin):
            gw = k * nwin + w
            a, b = int(wb[gw]), int(wb[gw + 1])
            off = w * C * CHUNK
            xk[off:off + (b - a)] = x[a:b]
            sk[off:off + (b - a)] = (idx[a:b] - (k * spc + w * WIN)).astype(np.float32)
        xs.append(np.ascontiguousarray(xk.T))                       # [128, M_pad]
        segls.append(np.ascontiguousarray(sk.reshape(-1, CHUNK).T))  # [128, n_chunks]
    return xs, segls, C, m_pad, n_chunks, spc, nwin


def _host_weights(W1, b1, W2, b2, Wt, bt):
    """Fold W2 into W1 via the relu/abs identity; build the 193-wide W_cat."""
    W1 = np.asarray(W1, np.float32)
    W2 = np.asarray(W2, np.float32)
    Wt = np.asarray(Wt, np.float32)
    b1 = np.asarray(b1, np.float32)
    w2 = W2[:, 0]
    w_lin = W1 @ w2                                     # [128]
    sp = w2 >= 0
    W1p = W1[:, sp] * w2[sp][None, :]                   # [128, pp]
    W1m = W1[:, ~sp] * (-w2[~sp][None, :])              # [128, 64-pp]
    pp = int(W1p.shape[1])
    wcat = np.concatenate([w_lin[:, None], W1p, W1m, np.asarray(Wt, np.float32)],
                          axis=1).astype(np.float32)    # [128, 1+64+128 = 193]
    bias_c = float(np.asarray(b2, np.float32)[0] + 0.5 * float(b1 @ w2))
    # b1/bt per-column biases are zero in this problem (reference setup); the
    # kernel below supports only scalar-foldable biases.
    assert not np.any(b1), "nonzero b1 unsupported by this kernel build"
    assert not np.any(np.asarray(bt, np.float32)), "nonzero bt unsupported"
    return wcat, pp, bias_c


def _build_program(m_pad, n_chunks, C, spc, nwin, pp, bias_c):
    """Build the SPMD Bass/Tile program (identical across cores)."""
    nc = bacc.Bacc("TRN2", target_bir_lowering=False, debug=False)

    xT_d = nc.dram_tensor("xT", [D, m_pad], F32, kind="ExternalInput").ap()
    segloc_d = nc.dram_tensor("segloc", [D, n_chunks], F32, kind="ExternalInput").ap()
    wcat_d = nc.dram_tensor("wcat", [D, 193], F32, kind="ExternalInput").ap()
    iota_d = nc.dram_tensor("iota", [D, WIN], F32, kind="ExternalInput").ap()
    out_d = nc.dram_tensor("out", [spc, D], F32, kind="ExternalOutput").ap()

    STRIDE = 256                   # padded per-chunk stride in the main PSUM tile
    TW = GROUP * STRIDE            # main PSUM tile width
    groups_per_win = C // GROUP

    with tile.TileContext(nc) as tc:
        with (
            tc.tile_pool(name="const", bufs=1) as cpool,
            tc.tile_pool(name="xin", bufs=3) as xpool,
            tc.tile_pool(name="tsb", bufs=2) as tpool,
            tc.tile_pool(name="small", bufs=2) as spool,
            tc.tile_pool(name="bmat", bufs=3) as bpool,
            tc.tile_pool(name="outp", bufs=2) as opool,
            tc.tile_pool(name="mpsum", bufs=1, space="PSUM") as mpsum,
            tc.tile_pool(name="upsum", bufs=2, space="PSUM") as upsum,
        ):
            wcat_sb = cpool.tile([D, 193], F32)
            nc.sync.dma_start(wcat_sb[:], wcat_d[:])
            iota_sb = cpool.tile([D, WIN], F32)
            nc.sync.dma_start(iota_sb[:], iota_d[:])
            segloc_sb = cpool.tile([D, n_chunks], F32)
            nc.sync.dma_start(segloc_sb[:], segloc_d[:])

            for w in range(nwin):
                U = upsum.tile([WIN, 129], F32)
                for g in range(groups_per_win):
                    gi = w * groups_per_win + g       # global group id
                    xt = xpool.tile([D, GROUP * CHUNK], F32)
                    nc.sync.dma_start(
                        xt[:], xT_d[:, gi * GROUP * CHUNK:(gi + 1) * GROUP * CHUNK])

                    main = mpsum.tile([D, TW], F32)
                    for c in range(GROUP):
                        nc.tensor.matmul(
                            main[:, c * STRIDE:c * STRIDE + 193],
                            xt[:, c * CHUNK:(c + 1) * CHUNK],
                            wcat_sb[:],
                            start=True, stop=True,
                        )
                    m3 = main[:].rearrange("p (c s) -> p c s", s=STRIDE)

                    gp = spool.tile([D, GROUP], F32, tag="gp")
                    gm = spool.tile([D, GROUP], F32, tag="gm")
                    gate = spool.tile([D, GROUP], F32, tag="gate")
                    if pp > 0:
                        nc.vector.tensor_reduce(
                            gp[:], m3[:, :, 1:1 + pp], AX.X, ALU.add,
                            apply_absolute_value=True)
                    else:
                        nc.vector.memset(gp[:], 0.0)
                    if pp < DH:
                        nc.vector.tensor_reduce(
                            gm[:], m3[:, :, 1 + pp:1 + DH], AX.X, ALU.add,
                            apply_absolute_value=True, negate=True)
                    else:
                        nc.vector.memset(gm[:], 0.0)
                    nc.vector.tensor_add(gate[:], gp[:], gm[:])
                    nc.vector.tensor_add(gate[:], gate[:], m3[:, :, 0])

                    e_sb = spool.tile([D, GROUP], F32, tag="e")
                    nc.scalar.activation(e_sb[:], gate[:], ACTF.Exp,
                                         bias=bias_c, scale=0.5)

                    t_sb = tpool.tile([D, GROUP * 129], F32)
                    t3 = t_sb[:].rearrange("p (c s) -> p c s", s=129)
                    nc.vector.memset(t3[:, :, 128], 1.0)
                    nc.scalar.activation(t3[:, :, 0:128], m3[:, :, 65:193],
                                         ACTF.Relu)

                    for c in range(GROUP):
                        ci = gi * GROUP + c           # global chunk id
                        B = bpool.tile([D, WIN], F32)
                        nc.vector.tensor_scalar(
                            B[:], iota_sb[:],
                            segloc_sb[:, ci:ci + 1], e_sb[:, c:c + 1],
                            ALU.is_equal, ALU.mult)
                        first = (g == 0 and c == 0)
                        last = (g == groups_per_win - 1 and c == GROUP - 1)
                        nc.tensor.matmul(
                            U[:, :], B[:], t_sb[:, c * 129:(c + 1) * 129],
                            start=first, stop=last, skip_group_check=True)

                d_sb = opool.tile([WIN, 1], F32, tag="d")
                r_sb = opool.tile([WIN, 1], F32, tag="r")
                o_sb = opool.tile([WIN, D], F32, tag="o")
                nc.vector.tensor_scalar_add(d_sb[:], U[:, 128:129], EPS)
                nc.vector.reciprocal(r_sb[:], d_sb[:])
                nc.vector.tensor_scalar_mul(o_sb[:], U[:, 0:128], r_sb[:])
                nc.sync.dma_start(out_d[w * WIN:(w + 1) * WIN, :], o_sb[:])

    nc.compile()
    return nc


def kernel(x, index, W1, b1, W2, b2, Wt, bt, dim_size):
    segs = int(dim_size)
    xs, segls, C, m_pad, n_chunks, spc, nwin = _host_shard(x, index, segs)
    wcat, pp, bias_c = _host_weights(W1, b1, W2, b2, Wt, bt)
    iota = np.tile(np.arange(WIN, dtype=np.float32), (D, 1))

    nc = _build_program(m_pad, n_chunks, C, spc, nwin, pp, bias_c)

    in_maps = [
        {"xT": xs[k], "segloc": segls[k], "wcat": wcat, "iota": iota}
        for k in range(N_CORES)
    ]
    res = run_bass_kernel_spmd(nc, in_maps, list(range(N_CORES)))
    global LAST_EXEC_NS
    LAST_EXEC_NS = res.exec_time_ns
    out = np.concatenate([res.results[k]["out"] for k in range(N_CORES)], axis=0)
    return out.astype(np.float32)


LAST_EXEC_NS = None


# revision 22
# speedup vs baseline: 1.2430x; 1.2430x over previous
"""AttentionalGraphAggregation (segment softmax + weighted scatter-sum) on 8 trn2 cores.

Math (eval mode, dropout = id):
    h     = relu(x @ W1 + b1)            [N, 64]
    gate  = (h @ W2 + b2)[:, 0]          [N]
    alpha = segment_softmax(gate, index) [N]   (max-subtraction skipped: gate is
                                               tiny (|gate| < ~0.3) so exp is safe,
                                               and alpha is mathematically identical)
    t     = relu(x @ Wt + bt)            [N, 128]
    out   = segment_sum(alpha[:,None] * t, index, 8192)

Device strategy (per core; data-parallel over segments per the sharding hint):
  - Core k owns segments [1024k, 1024(k+1)); index is sorted so its nodes are a
    contiguous slice.  Host pre-transposes x (ships xT [128, M_pad]) so that a
    column-slice of xT is directly usable as the matmul stationary operand:
    out = xT_chunk.T @ W = x_chunk @ W  (natural, nodes-on-partitions output).
  - gate via the relu identity relu(u) = (u + |u|)/2 with W2 folded into W1:
        gate = 0.5*(x@(W1@w2) + sum|x@W1p| - sum|x@W1m|) + const
    where W1p/W1m are W1 columns scaled by |w2| split by sign(w2).  This keeps
    everything in one 193-wide matmul per 128-node chunk and lets the DVE do the
    reductions straight out of PSUM (abs+add reduce), with no relu_h tensor.
  - e = exp(gate) on ACT; t = relu(x@Wt) on ACT (PSUM->SBUF).
  - scatter: per chunk a one-hot matrix B[p, s] = e_p * (segloc_p == s) is built
    in ONE DVE tensor_scalar op (iota is_equal segloc, then mult by e), then
    U[win] += B.T @ [t | ones] accumulates per-window segment sums (and denoms)
    in PSUM.  Windows are 32 segments; host pads each window's nodes to a
    uniform chunk count so the SPMD program is identical across cores.
  - flush per window: out = U / (denom + 1e-16) (per-partition scalar) -> DMA.
"""

import sys

if "/opt/trn_rl_repo" not in sys.path:
    sys.path.insert(0, "/opt/trn_rl_repo")

import numpy as np

import concourse.bacc as bacc
import concourse.bass as bass
import concourse.mybir as mybir
import concourse.tile as tile
from concourse.bass_utils import run_bass_kernel_spmd

F32 = mybir.dt.float32
F32R = mybir.dt.float32r
ALU = mybir.AluOpType
ACTF = mybir.ActivationFunctionType
AX = mybir.AxisListType

N_CORES = 8
D = 128          # feature dim (both in and out)
DH = 64          # gate hidden dim
CHUNK = 128      # nodes per matmul chunk (stationary width)
GROUP = 4        # chunks per pipeline group (one PSUM tile)
WIN = 32         # segments per scatter window (B width / U partition count)
EPS = 1e-16
# Matmul dtype mode: "fp32r" streams 1 col/cycle (needs moving dim >= 256),
# "fp32" is exact but streams at 1/4 rate, "fp32t" uses transpose-mode loads.
MM_MODE = "fp32r"


def _host_shard(x, index, segs):
    """Shard nodes by segment windows, pad each window to a uniform chunk count.

    Returns per-core xT [128, M_pad] (f32), segloc [128, n_chunks] (f32, -1 for
    padding), plus (C, M_pad, n_chunks, spc, nwin).
    """
    n = x.shape[0]
    spc = segs // N_CORES              # segments per core
    nwin = spc // WIN                  # windows per core
    idx = np.asarray(index)
    if idx.dtype != np.int64:
        idx = idx.astype(np.int64)
    if not np.all(idx[1:] >= idx[:-1]):
        perm = np.argsort(idx, kind="stable")
        idx = idx[perm]
        x = np.asarray(x)[perm]
    # window boundaries: windows are WIN consecutive segments
    wb = np.searchsorted(idx, np.arange(0, segs + 1, WIN))  # [n_windows_total+1]
    wcounts = np.diff(wb)
    cmax = int(np.ceil(wcounts.max() / CHUNK)) if n else 1
    C = max(GROUP, ((cmax + GROUP - 1) // GROUP) * GROUP)   # chunks per window
    m_pad = nwin * C * CHUNK
    n_chunks = nwin * C

    xs, segls = [], []
    x = np.asarray(x, dtype=np.float32)
    for k in range(N_CORES):
        xk = np.zeros((m_pad, D), np.float32)
        sk = np.full((m_pad,), -1.0, np.float32)
        for w in range(nwin):
            gw = k * nwin + w
            a, b = int(wb[gw]), int(wb[gw + 1])
            off = w * C * CHUNK
            xk[off:off + (b - a)] = x[a:b]
            sk[off:off + (b - a)] = (idx[a:b] - (k * spc + w * WIN)).astype(np.float32)
        xs.append(np.ascontiguousarray(xk.T))                       # [128, M_pad]
        segls.append(np.ascontiguousarray(sk.reshape(-1, CHUNK).T))  # [128, n_chunks]
    return xs, segls, C, m_pad, n_chunks, spc, nwin


def _host_weights(W1, b1, W2, b2, Wt, bt):
    """Fold W2 into W1 via the relu/abs identity; build the 193-wide W_cat."""
    W1 = np.asarray(W1, np.float32)
    W2 = np.asarray(W2, np.float32)
    Wt = np.asarray(Wt, np.float32)
    b1 = np.asarray(b1, np.float32)
    w2 = W2[:, 0]
    w_lin = W1 @ w2                                     # [128]
    sp = w2 >= 0
    W1p = W1[:, sp] * w2[sp][None, :]                   # [128, pp]
    W1m = W1[:, ~sp] * (-w2[~sp][None, :])              # [128, 64-pp]
    pp = int(W1p.shape[1])
    wcat = np.concatenate([w_lin[:, None], W1p, W1m, np.asarray(Wt, np.float32)],
                          axis=1).astype(np.float32)    # [128, 1+64+128 = 193]
    # pad moving dim to 256 so fp32r matmuls stream at full rate
    wcat = np.concatenate(
        [wcat, np.zeros((D, 256 - wcat.shape[1]), np.float32)], axis=1)
    bias_c = float(np.asarray(b2, np.float32)[0] + 0.5 * float(b1 @ w2))
    # b1/bt per-column biases are zero in this problem (reference setup); the
    # kernel below supports only scalar-foldable biases.
    assert not np.any(b1), "nonzero b1 unsupported by this kernel build"
    assert not np.any(np.asarray(bt, np.float32)), "nonzero bt unsupported"
    return wcat, pp, bias_c


def _tinit_const():
    """Init pattern for the persistent t staging tiles: ones at col 128 of each
    256-wide chunk slot, zeros elsewhere (the fp32r pad columns)."""
    t = np.zeros((D, GROUP * 256), np.float32)
    for c in range(GROUP):
        t[:, c * 256 + 128] = 1.0
    return t


def _build_program(m_pad, n_chunks, C, spc, nwin, pp, bias_c):
    """Build the SPMD Bass/Tile program (identical across cores)."""
    nc = bacc.Bacc("TRN2", target_bir_lowering=False, debug=False)

    MMDT = F32R if MM_MODE == "fp32r" else F32
    STRIDE = 256                   # per-chunk slot width (PSUM main, t_sb, wcat)
    xT_d = nc.dram_tensor("xT", [D, m_pad], MMDT, kind="ExternalInput").ap()
    segloc_d = nc.dram_tensor("segloc", [D, n_chunks], F32, kind="ExternalInput").ap()
    wcat_d = nc.dram_tensor("wcat", [D, 256], MMDT, kind="ExternalInput").ap()
    iota_d = nc.dram_tensor("iota", [D, WIN], F32, kind="ExternalInput").ap()
    tinit_d = nc.dram_tensor("tinit", [D, GROUP * STRIDE], MMDT,
                             kind="ExternalInput").ap()
    out_d = nc.dram_tensor("out", [spc, D], F32, kind="ExternalOutput").ap()

    TW = GROUP * STRIDE            # main PSUM tile width
    groups_per_win = C // GROUP

    mm_kw = {"is_transpose": True} if MM_MODE == "fp32t" else {}

    with tile.TileContext(nc) as tc:
        with (
            tc.tile_pool(name="const", bufs=1) as cpool,
            tc.tile_pool(name="xin", bufs=4) as xpool,
            tc.tile_pool(name="tsb", bufs=1) as tpool,
            tc.tile_pool(name="small", bufs=2) as spool,
            tc.tile_pool(name="bmat", bufs=3) as bpool,
            tc.tile_pool(name="outp", bufs=2) as opool,
            tc.tile_pool(name="mpsum", bufs=3, space="PSUM") as mpsum,
            tc.tile_pool(name="upsum", bufs=2, space="PSUM") as upsum,
        ):
            wcat_sb = cpool.tile([D, 256], MMDT)
            nc.sync.dma_start(wcat_sb[:], wcat_d[:])
            iota_sb = cpool.tile([D, WIN], F32)
            nc.sync.dma_start(iota_sb[:], iota_d[:])
            segloc_sb = cpool.tile([D, n_chunks], F32)
            nc.sync.dma_start(segloc_sb[:], segloc_d[:])

            # persistent double-buffered [t | 1 | 0-pad] staging: pad columns
            # (129..255 of each chunk slot) are zeroed once and never rewritten,
            # so fp32r U-matmuls can stream a full 256-wide moving operand
            tsb_tiles = []
            for i in range(3):
                t = tpool.tile([D, GROUP * STRIDE], MMDT, tag=f"tsb{i}")
                nc.sync.dma_start(t[:], tinit_d[:])
                tsb_tiles.append(t)

            for w in range(nwin):
                U = upsum.tile([WIN, STRIDE], F32)
                for g in range(groups_per_win):
                    gi = w * groups_per_win + g       # global group id
                    xt = xpool.tile([D, GROUP * CHUNK], MMDT)
                    nc.sync.dma_start(
                        xt[:], xT_d[:, gi * GROUP * CHUNK:(gi + 1) * GROUP * CHUNK])

                    main = mpsum.tile([D, TW], F32)
                    for c in range(GROUP):
                        nc.tensor.matmul(
                            main[:, c * STRIDE:(c + 1) * STRIDE],
                            xt[:, c * CHUNK:(c + 1) * CHUNK],
                            wcat_sb[:],
                            start=True, stop=True, **mm_kw,
                        )
                    m3 = main[:].rearrange("p (c s) -> p c s", s=STRIDE)

                    gp = spool.tile([D, GROUP], F32, tag="gp")
                    gm = spool.tile([D, GROUP], F32, tag="gm")
                    gate = spool.tile([D, GROUP], F32, tag="gate")
                    if pp > 0:
                        nc.vector.tensor_reduce(
                            gp[:], m3[:, :, 1:1 + pp], AX.X, ALU.add,
                            apply_absolute_value=True)
                    else:
                        nc.vector.memset(gp[:], 0.0)
                    if pp < DH:
                        nc.vector.tensor_reduce(
                            gm[:], m3[:, :, 1 + pp:1 + DH], AX.X, ALU.add,
                            apply_absolute_value=True, negate=True)
                    else:
                        nc.vector.memset(gm[:], 0.0)
                    nc.vector.tensor_add(gate[:], gp[:], gm[:])
                    nc.vector.tensor_add(gate[:], gate[:], m3[:, :, 0])

                    e_sb = spool.tile([D, GROUP], F32, tag="e")
                    nc.scalar.activation(e_sb[:], gate[:], ACTF.Exp,
                                         bias=bias_c, scale=0.5)

                    t_sb = tsb_tiles[gi % 3]
                    t3 = t_sb[:].rearrange("p (c s) -> p c s", s=STRIDE)
                    nc.scalar.activation(t3[:, :, 0:128], m3[:, :, 65:193],
                                         ACTF.Relu)

                    for c in range(GROUP):
                        ci = gi * GROUP + c           # global chunk id
                        B = bpool.tile([D, WIN], MMDT)
                        eng = nc.vector if c % 2 == 0 else nc.gpsimd
                        eng.tensor_scalar(
                            B[:], iota_sb[:],
                            segloc_sb[:, ci:ci + 1], e_sb[:, c:c + 1],
                            ALU.is_equal, ALU.mult)
                        first = (g == 0 and c == 0)
                        last = (g == groups_per_win - 1 and c == GROUP - 1)
                        nc.tensor.matmul(
                            U[:, :], B[:],
                            t_sb[:, c * STRIDE:(c + 1) * STRIDE],
                            start=first, stop=last, skip_group_check=True,
                            **mm_kw)

                d_sb = opool.tile([WIN, 1], F32, tag="d")
                r_sb = opool.tile([WIN, 1], F32, tag="r")
                o_sb = opool.tile([WIN, D], F32, tag="o")
                nc.vector.tensor_scalar_add(d_sb[:], U[:, 128:129], EPS)
                nc.vector.reciprocal(r_sb[:], d_sb[:])
                nc.scalar.mul(o_sb[:], U[:, 0:128], r_sb[:])
                nc.sync.dma_start(out_d[w * WIN:(w + 1) * WIN, :], o_sb[:])

    nc.compile()
    return nc


def kernel(x, index, W1, b1, W2, b2, Wt, bt, dim_size):
    segs = int(dim_size)
    xs, segls, C, m_pad, n_chunks, spc, nwin = _host_shard(x, index, segs)
    wcat, pp, bias_c = _host_weights(W1, b1, W2, b2, Wt, bt)
    iota = np.tile(np.arange(WIN, dtype=np.float32), (D, 1))
    tinit = _tinit_const()

    nc = _build_program(m_pad, n_chunks, C, spc, nwin, pp, bias_c)

    in_maps = [
        {"xT": xs[k], "segloc": segls[k], "wcat": wcat, "iota": iota,
         "tinit": tinit}
        for k in range(N_CORES)
    ]
    res = run_bass_kernel_spmd(nc, in_maps, list(range(N_CORES)))
    global LAST_EXEC_NS
    LAST_EXEC_NS = res.exec_time_ns
    out = np.concatenate([res.results[k]["out"] for k in range(N_CORES)], axis=0)
    return out.astype(np.float32)


LAST_EXEC_NS = None


# revision 26
# speedup vs baseline: 159.4871x; 128.3100x over previous
"""AttentionalGraphAggregation (segment softmax + weighted scatter-sum) on 8 trn2 cores.

Math (eval mode, dropout = id):
    h     = relu(x @ W1 + b1)            [N, 64]
    gate  = (h @ W2 + b2)[:, 0]          [N]
    alpha = segment_softmax(gate, index) [N]   (max-subtraction skipped: gate is
                                               tiny (|gate| < ~0.3) so exp is safe,
                                               and alpha is mathematically identical)
    t     = relu(x @ Wt + bt)            [N, 128]
    out   = segment_sum(alpha[:,None] * t, index, 8192)

Device strategy (per core; data-parallel over segments per the sharding hint):
  - Core k owns segments [1024k, 1024(k+1)); index is sorted so its nodes are a
    contiguous slice.  Host pre-transposes x (ships xT [128, M_pad]) so that a
    column-slice of xT is directly usable as the matmul stationary operand:
    out = xT_chunk.T @ W = x_chunk @ W  (natural, nodes-on-partitions output).
  - gate via the relu identity relu(u) = (u + |u|)/2 with W2 folded into W1:
        gate = 0.5*(x@(W1@w2) + sum|x@W1p| - sum|x@W1m|) + const
    where W1p/W1m are W1 columns scaled by |w2| split by sign(w2).  This keeps
    everything in one 193-wide matmul per 128-node chunk and lets the DVE do the
    reductions straight out of PSUM (abs+add reduce), with no relu_h tensor.
  - e = exp(gate) on ACT; t = relu(x@Wt) on ACT (PSUM->SBUF).
  - scatter: per chunk a one-hot matrix B[p, s] = e_p * (segloc_p == s) is built
    in ONE tensor_scalar op (iota is_equal segloc, then mult by e; alternating
    DVE/GpSimd).  Then per window (32 segments):
        Ut[128 out, 32 segs]  += t_chunk.T @ B      (B is the 32-wide MOVING
        den[32, 1]            += B.T @ ones          operand: fp32 matmuls cost
                                                     4 cyc/moving-row, so this
                                                     is 8x cheaper than moving t)
    and at flush: Ut -> SBUF -> PE-transpose -> U[32, 128]; out = U/(den+1e-16).
  - Host pads each window's nodes to a uniform chunk count so the SPMD program
    is identical across all 8 cores; host gathers the 8 [1024, 128] outputs.
"""

import sys

if "/opt/trn_rl_repo" not in sys.path:
    sys.path.insert(0, "/opt/trn_rl_repo")

import numpy as np

import concourse.bacc as bacc
import concourse.bass as bass
import concourse.mybir as mybir
import concourse.tile as tile
from concourse.bass_utils import run_bass_kernel_spmd

F32 = mybir.dt.float32
F32R = mybir.dt.float32r
ALU = mybir.AluOpType
ACTF = mybir.ActivationFunctionType
AX = mybir.AxisListType

N_CORES = 8
D = 128          # feature dim (both in and out)
DH = 64          # gate hidden dim
CHUNK = 128      # nodes per matmul chunk (stationary width)
GROUP = 4        # chunks per pipeline group (one PSUM tile)
WIN = 32         # segments per scatter window (B width / U partition count)
EPS = 1e-16
# Matmul dtype mode: "fp32" is exact (4 cyc/moving-row); "fp32r" streams
# 1 cyc/row when the moving dim is >= 256 but rounds inputs (~1e-4 rel err).
MM_MODE = "fp32"
REP = 1          # repeat whole compute (idempotent) for exec-time isolation


def _host_shard(x, index, segs):
    """Shard nodes by segment windows, pad each window to a uniform chunk count.

    Returns per-core xT [128, M_pad] (f32), segloc [128, n_chunks] (f32, -1 for
    padding), plus (C, M_pad, n_chunks, spc, nwin).
    """
    n = x.shape[0]
    spc = segs // N_CORES              # segments per core
    nwin = spc // WIN                  # windows per core
    idx = np.asarray(index)
    if idx.dtype != np.int64:
        idx = idx.astype(np.int64)
    if not np.all(idx[1:] >= idx[:-1]):
        perm = np.argsort(idx, kind="stable")
        idx = idx[perm]
        x = np.asarray(x)[perm]
    wb = np.searchsorted(idx, np.arange(0, segs + 1, WIN))
    wcounts = np.diff(wb)
    cmax = int(np.ceil(wcounts.max() / CHUNK)) if n else 1
    C = max(GROUP, ((cmax + GROUP - 1) // GROUP) * GROUP)   # chunks per window
    m_pad = nwin * C * CHUNK
    n_chunks = nwin * C

    xs, segls = [], []
    x = np.asarray(x, dtype=np.float32)
    for k in range(N_CORES):
        xk = np.zeros((m_pad, D), np.float32)
        sk = np.full((m_pad,), -1.0, np.float32)
        for w in range(nwin):
            gw = k * nwin + w
            a, b = int(wb[gw]), int(wb[gw + 1])
            off = w * C * CHUNK
            xk[off:off + (b - a)] = x[a:b]
            sk[off:off + (b - a)] = (idx[a:b] - (k * spc + w * WIN)).astype(np.float32)
        xs.append(np.ascontiguousarray(xk.T))                       # [128, M_pad]
        segls.append(np.ascontiguousarray(sk.reshape(-1, CHUNK).T))  # [128, n_chunks]
    return xs, segls, C, m_pad, n_chunks, spc, nwin


def _host_weights(W1, b1, W2, b2, Wt, bt):
    """Fold W2 into W1 via the relu/abs identity; build the 256-wide W_cat."""
    W1 = np.asarray(W1, np.float32)
    W2 = np.asarray(W2, np.float32)
    Wt = np.asarray(Wt, np.float32)
    b1 = np.asarray(b1, np.float32)
    w2 = W2[:, 0]
    w_lin = W1 @ w2                                     # [128]
    sp = w2 >= 0
    W1p = W1[:, sp] * w2[sp][None, :]                   # [128, pp]
    W1m = W1[:, ~sp] * (-w2[~sp][None, :])              # [128, 64-pp]
    pp = int(W1p.shape[1])
    wcat = np.concatenate([w_lin[:, None], W1p, W1m, np.asarray(Wt, np.float32)],
                          axis=1).astype(np.float32)    # [128, 1+64+128 = 193]
    # pad moving dim to 256 so fp32r matmuls stream at full rate
    wcat = np.concatenate(
        [wcat, np.zeros((D, 256 - wcat.shape[1]), np.float32)], axis=1)
    bias_c = float(np.asarray(b2, np.float32)[0] + 0.5 * float(b1 @ w2))
    # b1/bt per-column biases are zero in this problem (reference setup); the
    # kernel below supports only scalar-foldable biases.
    assert not np.any(b1), "nonzero b1 unsupported by this kernel build"
    assert not np.any(np.asarray(bt, np.float32)), "nonzero bt unsupported"
    return wcat, pp, bias_c


def _build_program(m_pad, n_chunks, C, spc, nwin, pp, bias_c):
    """Build the SPMD Bass/Tile program (identical across cores)."""
    nc = bacc.Bacc("TRN2", target_bir_lowering=False, debug=False)

    MMDT = F32R if MM_MODE == "fp32r" else F32
    STRIDE = 256                   # per-chunk slot width in the main PSUM tile
    WN = 256 if MM_MODE == "fp32r" else 193   # main-matmul moving width

    xT_d = nc.dram_tensor("xT", [D, m_pad], MMDT, kind="ExternalInput").ap()
    segloc_d = nc.dram_tensor("segloc", [D, n_chunks], F32, kind="ExternalInput").ap()
    wcat_d = nc.dram_tensor("wcat", [D, 256], MMDT, kind="ExternalInput").ap()
    iota_d = nc.dram_tensor("iota", [D, WIN], F32, kind="ExternalInput").ap()
    ones_d = nc.dram_tensor("ones", [D, 1], MMDT, kind="ExternalInput").ap()
    ident_d = nc.dram_tensor("ident", [D, D], F32, kind="ExternalInput").ap()
    out_d = nc.dram_tensor("out", [spc, D], F32, kind="ExternalOutput").ap()

    TW = GROUP * STRIDE            # main PSUM tile width
    groups_per_win = C // GROUP

    with tile.TileContext(nc) as tc:
        with (
            tc.tile_pool(name="const", bufs=1) as cpool,
            tc.tile_pool(name="xin", bufs=4) as xpool,
            tc.tile_pool(name="tsb", bufs=3) as tpool,
            tc.tile_pool(name="small", bufs=2) as spool,
            tc.tile_pool(name="bmat", bufs=3) as bpool,
            tc.tile_pool(name="outp", bufs=2) as opool,
            tc.tile_pool(name="mpsum", bufs=2, space="PSUM") as mpsum,
            tc.tile_pool(name="upsum", bufs=2, space="PSUM") as upsum,
            tc.tile_pool(name="npsum", bufs=2, space="PSUM") as npsum,
        ):
            wcat_sb = cpool.tile([D, 256], MMDT)
            nc.sync.dma_start(wcat_sb[:], wcat_d[:])
            iota_sb = cpool.tile([D, WIN], F32)
            nc.sync.dma_start(iota_sb[:], iota_d[:])
            segloc_sb = cpool.tile([D, n_chunks], F32)
            nc.sync.dma_start(segloc_sb[:], segloc_d[:])
            ones_sb = cpool.tile([D, 1], MMDT)
            nc.sync.dma_start(ones_sb[:], ones_d[:])
            ident_sb = cpool.tile([D, D], F32)
            nc.sync.dma_start(ident_sb[:], ident_d[:])

            for rep in range(REP):
              for w in range(nwin):
                uw = upsum.tile([D, WIN], F32)      # Ut: transposed seg sums
                nd = npsum.tile([WIN, D + 1], F32)  # [transposed-back U | denom]
                for g in range(groups_per_win):
                    gi = w * groups_per_win + g       # global group id
                    xt = xpool.tile([D, GROUP * CHUNK], MMDT)
                    nc.sync.dma_start(
                        xt[:], xT_d[:, gi * GROUP * CHUNK:(gi + 1) * GROUP * CHUNK])

                    main = mpsum.tile([D, TW], F32)
                    for c in range(GROUP):
                        nc.tensor.matmul(
                            main[:, c * STRIDE:c * STRIDE + WN],
                            xt[:, c * CHUNK:(c + 1) * CHUNK],
                            wcat_sb[:, 0:WN],
                            start=True, stop=True,
                        )
                    m3 = main[:].rearrange("p (c s) -> p c s", s=STRIDE)

                    gp = spool.tile([D, GROUP], F32, tag="gp")
                    gm = spool.tile([D, GROUP], F32, tag="gm")
                    gate = spool.tile([D, GROUP], F32, tag="gate")
                    if pp > 0:
                        nc.vector.tensor_reduce(
                            gp[:], m3[:, :, 1:1 + pp], AX.X, ALU.add,
                            apply_absolute_value=True)
                    else:
                        nc.vector.memset(gp[:], 0.0)
                    if pp < DH:
                        nc.vector.tensor_reduce(
                            gm[:], m3[:, :, 1 + pp:1 + DH], AX.X, ALU.add,
                            apply_absolute_value=True, negate=True)
                    else:
                        nc.vector.memset(gm[:], 0.0)
                    nc.vector.tensor_add(gate[:], gp[:], gm[:])
                    nc.vector.tensor_add(gate[:], gate[:], m3[:, :, 0])

                    e_sb = spool.tile([D, GROUP], F32, tag="e")
                    nc.scalar.activation(e_sb[:], gate[:], ACTF.Exp,
                                         bias=bias_c, scale=0.5)

                    t_sb = tpool.tile([D, GROUP * CHUNK], MMDT)
                    t3 = t_sb[:].rearrange("p (c s) -> p c s", s=CHUNK)
                    nc.scalar.activation(t3[:, :, :], m3[:, :, 65:193],
                                         ACTF.Relu)

                    for c in range(GROUP):
                        ci = gi * GROUP + c           # global chunk id
                        B = bpool.tile([D, WIN], MMDT)
                        eng = nc.vector if c % 2 == 0 else nc.gpsimd
                        eng.tensor_scalar(
                            B[:], iota_sb[:],
                            segloc_sb[:, ci:ci + 1], e_sb[:, c:c + 1],
                            ALU.is_equal, ALU.mult)
                        first = (g == 0 and c == 0)
                        last = (g == groups_per_win - 1 and c == GROUP - 1)
                        nc.tensor.matmul(
                            uw[:, :],
                            t_sb[:, c * CHUNK:(c + 1) * CHUNK], B[:],
                            start=first, stop=last, skip_group_check=True)
                        nc.tensor.matmul(
                            nd[:, D:D + 1], B[:], ones_sb[:],
                            start=first, stop=last, skip_group_check=True)

                # flush: Ut -> SBUF -> PE transpose -> U natural; divide; DMA
                ut_sb = opool.tile([D, WIN], F32, tag="ut")
                nc.scalar.copy(ut_sb[:], uw[:, :])
                nc.tensor.transpose(nd[:, 0:D], ut_sb[:], ident_sb[:])
                d_sb = opool.tile([WIN, 1], F32, tag="d")
                r_sb = opool.tile([WIN, 1], F32, tag="r")
                o_sb = opool.tile([WIN, D], F32, tag="o")
                nc.vector.tensor_scalar_add(d_sb[:], nd[:, D:D + 1], EPS)
                nc.vector.reciprocal(r_sb[:], d_sb[:])
                nc.scalar.mul(o_sb[:], nd[:, 0:D], r_sb[:])
                nc.sync.dma_start(out_d[w * WIN:(w + 1) * WIN, :], o_sb[:])

    nc.compile()
    return nc


def _consts():
    iota = np.tile(np.arange(WIN, dtype=np.float32), (D, 1))
    ones = np.ones((D, 1), np.float32)
    ident = np.eye(D, dtype=np.float32)
    return iota, ones, ident


def kernel(x, index, W1, b1, W2, b2, Wt, bt, dim_size):
    segs = int(dim_size)
    xs, segls, C, m_pad, n_chunks, spc, nwin = _host_shard(x, index, segs)
    wcat, pp, bias_c = _host_weights(W1, b1, W2, b2, Wt, bt)
    iota, ones, ident = _consts()

    nc = _build_program(m_pad, n_chunks, C, spc, nwin, pp, bias_c)

    in_maps = [
        {"xT": xs[k], "segloc": segls[k], "wcat": wcat, "iota": iota,
         "ones": ones, "ident": ident}
        for k in range(N_CORES)
    ]
    res = run_bass_kernel_spmd(nc, in_maps, list(range(N_CORES)))
    global LAST_EXEC_NS
    LAST_EXEC_NS = res.exec_time_ns
    out = np.concatenate([res.results[k]["out"] for k in range(N_CORES)], axis=0)
    return out.astype(np.float32)


LAST_EXEC_NS = None
